# revision 9
# baseline (speedup 1.0000x reference)
"""CrossAttentionBlock kernel for 8 Trainium2 NeuronCores.

Data-parallel over batch (B=8 -> one batch element per core). Each core
computes the full block for its element:
    q = freq @ Wq + bq ; k = img @ Wk + bk ; v = img @ Wv + bv   (12 heads, hd=64)
    ctx = softmax(q k^T / 8) v
    h = freq + LN(ctx) ; out = h + LN(relu(h @ W1 + b1) @ W2 + b2)

On-device everything lives in a transposed ("T") layout: features on the
SBUF partition dim, tokens on the free dim, so chained matmuls need no
transposes. Softmax runs on scores^T (k on partitions): the column of ones
appended to each head of V turns the softmax denominator into one extra
output partition of the ctx matmul. exp() needs no max-subtraction: logits
are sum_64 of O(0.3)-scale products / 8 (|logit| < ~2 for any plausible
input drawn at this scale). LayerNorm reductions over features (the
partition dim) are ones-vector matmuls on the PE; per-token stats are
broadcast back across partitions with a rank-1 ones matmul.

All matmuls run in float32r (full-rate fp32 on the PE, ~1e-4 rel err).
"""

import numpy as np

import concourse.bacc as bacc
import concourse.tile as tile
from concourse import mybir
from concourse.bass_utils import run_bass_kernel_spmd

F32 = mybir.dt.float32
F32R = mybir.dt.float32r
AF = mybir.ActivationFunctionType
OP = mybir.AluOpType

B = 8
SQ = 1024          # query tokens per core
SK = 1024          # kv tokens per core
H = 768            # hidden
NH = 12            # heads
HD = 64            # head dim
FF = 3072          # mlp intermediate
EPS = 1e-5
P = 128
NCT = H // P       # 6 c-tiles
NFT = FF // P      # 24 f-tiles
NKT = SK // P      # 8 k-tiles
QC = 512           # token chunk for LN/MLP
NQC = SQ // QC     # 2
AC = 256           # token chunk for attention
NAC = SQ // AC     # 4

# bias-pack column offsets (see _pack_biases)
_BQ, _BK, _BV, _B2, _G1, _BE1, _G2, _BE2, _B1 = 0, 6, 12, 18, 24, 30, 36, 42, 48
_BP_COLS = 48 + NFT


class _Ctx:
    pass


def _emit(nc, tc, t, reps=1, zero=None):
    cx = _Ctx()
    cx.zero = zero or {}
    consts = tc.alloc_tile_pool(name="consts", bufs=1)
    ring = tc.alloc_tile_pool(name="ring", bufs=5)
    wstream = tc.alloc_tile_pool(name="wstream", bufs=2)
    scratch = tc.alloc_tile_pool(name="scratch", bufs=2)
    stats = tc.alloc_tile_pool(name="stats", bufs=6)
    ps = tc.alloc_tile_pool(name="ps", bufs=2, space="PSUM")
    score_ps = tc.alloc_tile_pool(name="score_ps", bufs=2, space="PSUM")
    ctx_ps = tc.alloc_tile_pool(name="ctx_ps", bufs=2, space="PSUM")
    pools = [consts, ring, wstream, scratch, stats, ps, score_ps, ctx_ps]
    cx.ring, cx.wstream, cx.scratch, cx.stats = ring, wstream, scratch, stats
    cx.ps, cx.score_ps, cx.ctx_ps = ps, score_ps, ctx_ps

    # ---- constants -------------------------------------------------------
    bp = consts.tile([P, _BP_COLS], F32, name="bias_pack", tag="bp")
    nc.sync.dma_start(out=bp, in_=t["bias_pack"].ap())
    ones_col = consts.tile([P, 1], F32R, name="ones_col", tag="onec")
    nc.sync.dma_start(out=ones_col, in_=t["ones_col"].ap())
    ones_row = consts.tile([1, P], F32R, name="ones_row", tag="oner")
    nc.sync.dma_start(out=ones_row, in_=t["ones_row"].ap())
    eps_sb = consts.tile([1, 1], F32, name="eps_sb", tag="eps")
    nc.vector.memset(eps_sb, EPS)
    cx.bp, cx.ones_col, cx.ones_row, cx.eps = bp, ones_col, ones_row, eps_sb

    for _ in range(reps):
        _emit_block(nc, cx, t)

    for p in reversed(pools):
        p.release()


def _rtile(cx, shape, dtype, name):
    return cx.ring.tile(shape, dtype, name=name, tag="big")


def _emit_block(nc, cx, t):
    bp = cx.bp

    def bpc(base, i, lo=0, n=P):
        return cx.bp[lo:lo + n, base + i:base + i + 1]

    # ---- inputs ----------------------------------------------------------
    freqT = _rtile(cx, [P, NCT, SQ], F32R, "freqT")
    nc.sync.dma_start(out=freqT, in_=t["freqT"].ap().rearrange("(ci p) q -> p ci q", p=P))
    imgT = _rtile(cx, [P, NCT, SK], F32R, "imgT")
    nc.sync.dma_start(out=imgT, in_=t["imgT"].ap().rearrange("(ci p) q -> p ci q", p=P))

    # ---- QKV projections (V first so attention can start early) ----------
    # V[k,do] with a ones column per head: v[:, kt, h, 0:64]=V, [..., 64]=1
    v = _rtile(cx, [P, NKT, NH, HD + 1], F32R, "v")
    wv_sb = cx.wstream.tile([P, NCT, H], F32R, name="wv", tag="wv", bufs=1)
    nc.sync.dma_start(out=wv_sb, in_=t["wv"].ap().rearrange("(ci p) d -> p ci d", p=P))
    for kt in range(NKT):
        pa = cx.ps.tile([P, QC], F32, name="mm", tag="mm")
        pb = cx.ps.tile([P, QC], F32, name="mm", tag="mm")
        for ci in range(NCT):
            nc.tensor.matmul(pa, lhsT=imgT[:, ci, kt * P:(kt + 1) * P],
                             rhs=wv_sb[:, ci, :QC], start=(ci == 0), stop=(ci == NCT - 1))
            nc.tensor.matmul(pb[:, :H - QC], lhsT=imgT[:, ci, kt * P:(kt + 1) * P],
                             rhs=wv_sb[:, ci, QC:], start=(ci == 0), stop=(ci == NCT - 1))
        nc.scalar.activation(v[:, kt, 0:8, 0:HD],
                             pa.rearrange("p (h d) -> p h d", d=HD), AF.Copy)
        nc.scalar.activation(v[:, kt, 8:NH, 0:HD],
                             pb[:, :H - QC].rearrange("p (h d) -> p h d", d=HD), AF.Copy)
        nc.vector.tensor_copy(v[:, kt, :, HD], cx.ones_col.to_broadcast((P, NH)))

    # qT[do,q] = sum_c Wq[c,do] * freqT[c,q]   (kT likewise from imgT)
    qT = _rtile(cx, [P, NCT, SQ], F32R, "qT")
    kT = _rtile(cx, [P, NCT, SK], F32R, "kT")
    for nm, src, dst, bb in (("wkr", imgT, kT, _BK), ("wqr", freqT, qT, _BQ)):
        for di in range(NCT):
            slab = cx.wstream.tile([P, NCT, P], F32R, name="wqk", tag="wqk")
            nc.sync.dma_start(out=slab, in_=t[nm].ap()[di])
            for qc in range(NQC):
                pt = cx.ps.tile([P, QC], F32, name="mm", tag="mm")
                for ci in range(NCT):
                    nc.tensor.matmul(
                        pt, lhsT=slab[:, ci, :], rhs=src[:, ci, qc * QC:(qc + 1) * QC],
                        start=(ci == 0), stop=(ci == NCT - 1))
                dst_s = dst[:, di, qc * QC:(qc + 1) * QC]
                if cx.zero.get(bb, False):
                    nc.scalar.activation(dst_s, pt, AF.Copy)
                else:
                    nc.vector.tensor_scalar_add(dst_s, pt, bpc(bb, di))

    # ---- attention -------------------------------------------------------
    ctxT = _rtile(cx, [P, NCT, SQ], F32R, "ctxT")
    est = _rtile(cx, [P, 2, NKT, AC], F32R, "est")   # manual double buffer
    chunk = 0
    for h in range(NH):
        ci, lo = h // 2, (h % 2) * HD
        for ac in range(NAC):
            par = chunk % 2
            chunk += 1
            qs = qT[lo:lo + HD, ci, ac * AC:(ac + 1) * AC]
            for g in range(2):
                sp = cx.score_ps.tile([P, 4, AC], F32, name="sp", tag="sp")
                for j in range(4):
                    kt = 4 * g + j
                    nc.tensor.matmul(sp[:, j, :],
                                     lhsT=kT[lo:lo + HD, ci, kt * P:(kt + 1) * P],
                                     rhs=qs, start=True, stop=True)
                nc.scalar.activation(est[:, par, 4 * g:4 * g + 4, :], sp, AF.Exp)
            cp = cx.ctx_ps.tile([HD + 1, AC], F32, name="cp", tag="cp")
            for kt in range(NKT):
                nc.tensor.matmul(cp, lhsT=v[:, kt, h, :], rhs=est[:, par, kt, :],
                                 start=(kt == 0), stop=(kt == NKT - 1))
            rec = cx.stats.tile([1, AC], F32R, name="rec", tag="stat")
            with nc.allow_low_precision(reason="f32r recip feeds PE broadcast"):
                nc.vector.reciprocal(rec, cp[HD:HD + 1, :])
            rb = cx.ps.tile([HD, AC], F32, name="mm", tag="mm")
            nc.tensor.matmul(rb, lhsT=cx.ones_row[:, 0:HD], rhs=rec,
                             start=True, stop=True)
            cslice = ctxT[lo:lo + HD, ci, ac * AC:(ac + 1) * AC]
            nc.vector.tensor_copy(cslice, cp[0:HD, :])
            nc.vector.tensor_tensor(cslice, cslice, rb, OP.mult)
            if not cx.zero.get(_BV, False):
                nc.vector.tensor_scalar_add(cslice, cslice, bpc(_BV, ci, lo, HD))

    # ---- h = freqT + LN(ctxT) -------------------------------------------
    freqT2 = _rtile(cx, [P, NCT, SQ], F32R, "freqT2")
    nc.sync.dma_start(out=freqT2,
                      in_=t["freqT"].ap().rearrange("(ci p) q -> p ci q", p=P))
    hT = _rtile(cx, [P, NCT, SQ], F32R, "hT")
    for qc in range(NQC):
        _emit_ln_residual(nc, cx, ctxT[:, :, qc * QC:(qc + 1) * QC],
                          freqT2[:, :, qc * QC:(qc + 1) * QC],
                          hT[:, :, qc * QC:(qc + 1) * QC], _G1, _BE1)

    # ---- MLP + second LN -------------------------------------------------
    m1a = _rtile(cx, [P, NFT // 2, QC], F32R, "m1a")
    m1b = _rtile(cx, [P, NFT // 2, QC], F32R, "m1b")
    for qc in range(NQC):
        m2o = _rtile(cx, [P, 2, NCT, QC], F32R, "m2o")  # [0]=mlp2, [1]=out
        for fi in range(NFT):
            slab = cx.wstream.tile([P, NCT, P], F32R, name="w1", tag="w1")
            nc.sync.dma_start(out=slab, in_=t["w1r"].ap()[fi])
            m1 = m1a if fi < NFT // 2 else m1b
            pt = cx.ps.tile([P, QC], F32, name="mm", tag="mm")
            for ci in range(NCT):
                nc.tensor.matmul(pt, lhsT=slab[:, ci, :],
                                 rhs=hT[:, ci, qc * QC:(qc + 1) * QC],
                                 start=(ci == 0), stop=(ci == NCT - 1))
            # bias + relu on the (otherwise idle) ACT engine
            nc.scalar.activation(m1[:, fi % (NFT // 2), :], pt, AF.Relu,
                                 bias=bpc(_B1, fi))
        for ci in range(NCT):
            slab = cx.wstream.tile([P, NFT, P], F32R, name="w2", tag="w2")
            nc.sync.dma_start(out=slab, in_=t["w2r"].ap()[ci])
            pt = cx.ps.tile([P, QC], F32, name="mm", tag="mm")
            for fi in range(NFT):
                m1 = m1a if fi < NFT // 2 else m1b
                nc.tensor.matmul(pt, lhsT=slab[:, fi, :],
                                 rhs=m1[:, fi % (NFT // 2), :],
                                 start=(fi == 0), stop=(fi == NFT - 1))
            if cx.zero.get(_B2, False):
                nc.scalar.activation(m2o[:, 0, ci, :], pt, AF.Copy)
            else:
                nc.vector.tensor_scalar_add(m2o[:, 0, ci, :], pt, bpc(_B2, ci))
        _emit_ln_residual(nc, cx, m2o[:, 0], hT[:, :, qc * QC:(qc + 1) * QC],
                          m2o[:, 1], _G2, _BE2, res_on_pool=True)
        nc.sync.dma_start(
            out=t["outT"].ap().rearrange("(ci p) q -> p ci q", p=P)[:, :, qc * QC:(qc + 1) * QC],
            in_=m2o[:, 1])


def _emit_ln_residual(nc, cx, xs, rs, os_, gbase, bbase, res_on_pool=False):
    """os_ = rs + layernorm(xs)*g + b over features (partition x ci dim).

    xs/rs/os_: [P, NCT, W] APs (W tokens). Mean/var via ones-matmul partition
    reductions; per-token stats broadcast back with a rank-1 ones matmul.
    """
    W = xs.shape[2]
    bp = cx.bp
    sq = _rtile(cx, [P, NCT, W], F32R, "ln_sq")
    nc.vector.tensor_tensor(sq, xs, xs, OP.mult)
    s_ps = cx.ps.tile([1, W], F32, name="mm", tag="mm")
    for ci in range(NCT):
        nc.tensor.matmul(s_ps, lhsT=cx.ones_col, rhs=xs[:, ci, :],
                         start=(ci == 0), stop=(ci == NCT - 1))
    q_ps = cx.ps.tile([1, W], F32, name="mm", tag="mm")
    for ci in range(NCT):
        nc.tensor.matmul(q_ps, lhsT=cx.ones_col, rhs=sq[:, ci, :],
                         start=(ci == 0), stop=(ci == NCT - 1))
    m = cx.stats.tile([1, W], F32R, name="ln_m", tag="stat")
    nc.vector.tensor_scalar_mul(m, s_ps, 1.0 / H)
    var = cx.stats.tile([1, W], F32, name="ln_var", tag="stat")
    nc.vector.tensor_tensor(var, m, m, OP.mult)
    ex2 = cx.stats.tile([1, W], F32, name="ln_ex2", tag="stat")
    nc.vector.tensor_scalar_mul(ex2, q_ps, 1.0 / H)
    nc.vector.tensor_tensor(var, ex2, var, OP.subtract)
    std = cx.stats.tile([1, W], F32, name="ln_std", tag="stat")
    nc.scalar.activation(std, var, AF.Sqrt, bias=cx.eps)
    inv = cx.stats.tile([1, W], F32R, name="ln_inv", tag="stat")
    with nc.allow_low_precision(reason="f32r recip feeds PE broadcast"):
        nc.vector.reciprocal(inv, std)
    mb = cx.ps.tile([P, W], F32, name="mm", tag="mm")
    nc.tensor.matmul(mb, lhsT=cx.ones_row, rhs=m, start=True, stop=True)
    ib = cx.ps.tile([P, W], F32, name="mm", tag="mm")
    nc.tensor.matmul(ib, lhsT=cx.ones_row, rhs=inv, start=True, stop=True)
    plain = cx.zero.get(gbase, False) and cx.zero.get(bbase, False)
    for ci in range(NCT):
        tt = cx.scratch.tile([P, W], F32, name="lnt", tag="lnt")
        nc.vector.tensor_tensor(tt, xs[:, ci, :], mb, OP.subtract)
        nc.vector.tensor_tensor(tt, tt, ib, OP.mult)
        if not plain:
            nc.vector.tensor_scalar(out=tt, in0=tt,
                                    scalar1=bp[:, gbase + ci:gbase + ci + 1],
                                    scalar2=bp[:, bbase + ci:bbase + ci + 1],
                                    op0=OP.mult, op1=OP.add)
        eng = nc.gpsimd if res_on_pool else nc.vector
        eng.tensor_tensor(os_[:, ci, :], tt, rs[:, ci, :], OP.add)


def build_nc(reps=1, zero=None):
    nc = bacc.Bacc("TRN2", target_bir_lowering=False, debug=False)
    t = {}
    t["freqT"] = nc.declare_dram_parameter("freqT", [H, SQ], F32R, isOutput=False)
    t["imgT"] = nc.declare_dram_parameter("imgT", [H, SK], F32R, isOutput=False)
    t["wqr"] = nc.declare_dram_parameter("wqr", [NCT, P, NCT, P], F32R, isOutput=False)
    t["wkr"] = nc.declare_dram_parameter("wkr", [NCT, P, NCT, P], F32R, isOutput=False)
    t["wv"] = nc.declare_dram_parameter("wv", [H, H], F32R, isOutput=False)
    t["w1r"] = nc.declare_dram_parameter("w1r", [NFT, P, NCT, P], F32R, isOutput=False)
    t["w2r"] = nc.declare_dram_parameter("w2r", [NCT, P, NFT, P], F32R, isOutput=False)
    t["bias_pack"] = nc.declare_dram_parameter("bias_pack", [P, _BP_COLS], F32, isOutput=False)
    t["ones_col"] = nc.declare_dram_parameter("ones_col", [P, 1], F32R, isOutput=False)
    t["ones_row"] = nc.declare_dram_parameter("ones_row", [1, P], F32R, isOutput=False)
    t["outT"] = nc.declare_dram_parameter("outT", [H, SQ], F32R, isOutput=True)

    with tile.TileContext(nc) as tc:
        _emit(nc, tc, t, reps=reps, zero=zero)
    nc.compile()
    return nc


def _host_prep(inputs):
    f = np.ascontiguousarray(np.asarray(inputs["freq_hidden_states"], np.float32))
    im = np.ascontiguousarray(np.asarray(inputs["image_hidden_states"], np.float32))
    scale = np.float32(1.0 / np.sqrt(np.float32(HD)))
    wq = np.asarray(inputs["Wq"], np.float32) * scale
    bq = np.asarray(inputs["bq"], np.float32) * scale
    wk = np.asarray(inputs["Wk"], np.float32)
    wv = np.ascontiguousarray(np.asarray(inputs["Wv"], np.float32))
    w1 = np.asarray(inputs["W1"], np.float32)
    w2 = np.asarray(inputs["W2"], np.float32)

    def qk_tiles(w):  # [768,768] -> [di, p, ci, 128] holding lhsT tiles w[ci,di]
        return np.ascontiguousarray(w.reshape(NCT, P, NCT, P).transpose(2, 1, 0, 3))

    wqr = qk_tiles(wq)
    wkr = qk_tiles(wk)
    w1r = np.ascontiguousarray(w1.reshape(NCT, P, NFT, P).transpose(2, 1, 0, 3))
    w2r = np.ascontiguousarray(w2.reshape(NFT, P, NCT, P).transpose(2, 1, 0, 3))

    bpk = np.zeros((P, _BP_COLS), np.float32)
    for base, vec in ((_BQ, bq), (_BK, np.asarray(inputs["bk"], np.float32)),
                      (_BV, np.asarray(inputs["bv"], np.float32)),
                      (_B2, np.asarray(inputs["b2"], np.float32)),
                      (_G1, np.asarray(inputs["ln_att_g"], np.float32)),
                      (_BE1, np.asarray(inputs["ln_att_b"], np.float32)),
                      (_G2, np.asarray(inputs["ln_mlp_g"], np.float32)),
                      (_BE2, np.asarray(inputs["ln_mlp_b"], np.float32)),
                      (_B1, np.asarray(inputs["b1"], np.float32))):
        n = vec.shape[0] // P
        bpk[:, base:base + n] = vec.reshape(n, P).T
    shared = {
        "wqr": wqr, "wkr": wkr, "wv": wv, "w1r": w1r, "w2r": w2r,
        "bias_pack": bpk,
        "ones_col": np.ones((P, 1), np.float32),
        "ones_row": np.ones((1, P), np.float32),
    }
    in_maps = []
    for i in range(B):
        m = dict(shared)
        m["freqT"] = np.ascontiguousarray(f[i].T)
        m["imgT"] = np.ascontiguousarray(im[i].T)
        in_maps.append(m)
    return in_maps


_NC_CACHE = {}


def _zero_flags(inputs):
    def z(v):
        return bool(np.all(np.asarray(v) == 0))

    def one(v):
        return bool(np.all(np.asarray(v) == 1))

    return {
        _BQ: z(inputs["bq"]), _BK: z(inputs["bk"]), _BV: z(inputs["bv"]),
        _B2: z(inputs["b2"]),
        _G1: one(inputs["ln_att_g"]), _BE1: z(inputs["ln_att_b"]),
        _G2: one(inputs["ln_mlp_g"]), _BE2: z(inputs["ln_mlp_b"]),
    }


def kernel(**inputs) -> np.ndarray:
    in_maps = _host_prep(inputs)
    key = tuple(sorted(_zero_flags(inputs).items()))
    if _NC_CACHE.get("key") != key:
        _NC_CACHE["nc"] = build_nc(zero=_zero_flags(inputs))
        _NC_CACHE["key"] = key
    nc = _NC_CACHE["nc"]
    res = run_bass_kernel_spmd(nc, in_maps, core_ids=list(range(B)))
    out = np.stack([np.ascontiguousarray(r["outT"].T) for r in res.results])
    return out.astype(np.float32)


# revision 10
# speedup vs baseline: 1.2151x; 1.2151x over previous
"""CrossAttentionBlock kernel for 8 Trainium2 NeuronCores.

Data-parallel over batch (B=8 -> one batch element per core). Each core
computes the full block for its element:
    q = freq @ Wq + bq ; k = img @ Wk + bk ; v = img @ Wv + bv   (12 heads, hd=64)
    ctx = softmax(q k^T / 8) v
    h = freq + LN(ctx) ; out = h + LN(relu(h @ W1 + b1) @ W2 + b2)

On-device everything lives in a transposed ("T") layout: features on the
SBUF partition dim, tokens on the free dim, so chained matmuls need no
transposes. Softmax runs on scores^T (k on partitions): the column of ones
appended to each head of V turns the softmax denominator into one extra
output partition of the ctx matmul. exp() needs no max-subtraction: logits
are sum_64 of O(0.3)-scale products / 8 (|logit| < ~2 for any plausible
input drawn at this scale). LayerNorm reductions over features (the
partition dim) are ones-vector matmuls on the PE; per-token stats are
broadcast back across partitions with a rank-1 ones matmul.

All matmuls run in float32r (full-rate fp32 on the PE, ~1e-4 rel err).
"""

import numpy as np

import concourse.bacc as bacc
import concourse.tile as tile
from concourse import mybir
from concourse.bass_utils import run_bass_kernel_spmd

F32 = mybir.dt.float32
F32R = mybir.dt.float32r
BF16 = mybir.dt.bfloat16
AF = mybir.ActivationFunctionType
OP = mybir.AluOpType

B = 8
SQ = 1024          # query tokens per core
SK = 1024          # kv tokens per core
H = 768            # hidden
NH = 12            # heads
HD = 64            # head dim
FF = 3072          # mlp intermediate
EPS = 1e-5
P = 128
NCT = H // P       # 6 c-tiles
NFT = FF // P      # 24 f-tiles
NKT = SK // P      # 8 k-tiles
QC = 512           # token chunk for LN/MLP
NQC = SQ // QC     # 2
AC = 256           # token chunk for attention
NAC = SQ // AC     # 4

# bias-pack column offsets (see _pack_biases)
_BQ, _BK, _BV, _B2, _G1, _BE1, _G2, _BE2, _B1 = 0, 6, 12, 18, 24, 30, 36, 42, 48
_BP_COLS = 48 + NFT


class _Ctx:
    pass


def _emit(nc, tc, t, reps=1, zero=None):
    cx = _Ctx()
    cx.zero = zero or {}
    consts = tc.alloc_tile_pool(name="consts", bufs=1)
    ring = tc.alloc_tile_pool(name="ring", bufs=5)
    wstream = tc.alloc_tile_pool(name="wstream", bufs=2)
    scratch = tc.alloc_tile_pool(name="scratch", bufs=2)
    stats = tc.alloc_tile_pool(name="stats", bufs=6)
    ps = tc.alloc_tile_pool(name="ps", bufs=2, space="PSUM")
    score_ps = tc.alloc_tile_pool(name="score_ps", bufs=2, space="PSUM")
    ctx_ps = tc.alloc_tile_pool(name="ctx_ps", bufs=2, space="PSUM")
    pools = [consts, ring, wstream, scratch, stats, ps, score_ps, ctx_ps]
    cx.ring, cx.wstream, cx.scratch, cx.stats = ring, wstream, scratch, stats
    cx.ps, cx.score_ps, cx.ctx_ps = ps, score_ps, ctx_ps

    # ---- constants -------------------------------------------------------
    bp = consts.tile([P, _BP_COLS], F32, name="bias_pack", tag="bp")
    nc.sync.dma_start(out=bp, in_=t["bias_pack"].ap())
    ones_col = consts.tile([P, 1], F32R, name="ones_col", tag="onec")
    nc.sync.dma_start(out=ones_col, in_=t["ones_col"].ap())
    ones_row = consts.tile([1, P], F32R, name="ones_row", tag="oner")
    nc.sync.dma_start(out=ones_row, in_=t["ones_row"].ap())
    eps_sb = consts.tile([1, 1], F32, name="eps_sb", tag="eps")
    nc.vector.memset(eps_sb, EPS)
    cx.bp, cx.ones_col, cx.ones_row, cx.eps = bp, ones_col, ones_row, eps_sb

    for _ in range(reps):
        _emit_block(nc, cx, t)

    for p in reversed(pools):
        p.release()


def _rtile(cx, shape, dtype, name):
    return cx.ring.tile(shape, dtype, name=name, tag="big")


def _emit_block(nc, cx, t):
    bp = cx.bp

    def bpc(base, i, lo=0, n=P):
        return cx.bp[lo:lo + n, base + i:base + i + 1]

    # ---- inputs ----------------------------------------------------------
    freqT = _rtile(cx, [P, NCT, SQ], F32R, "freqT")
    nc.sync.dma_start(out=freqT, in_=t["freqT"].ap().rearrange("(ci p) q -> p ci q", p=P))
    imgT = _rtile(cx, [P, NCT, SK], F32R, "imgT")
    nc.sync.dma_start(out=imgT, in_=t["imgT"].ap().rearrange("(ci p) q -> p ci q", p=P))

    # ---- QKV projections (V first so attention can start early) ----------
    # V[k,do] with a ones column per head: v[:, kt, h, 0:64]=V, [..., 64]=1
    v = _rtile(cx, [P, NKT, NH, HD + 1], F32R, "v")
    wv_sb = cx.wstream.tile([P, NCT, H], F32R, name="wv", tag="wv", bufs=1)
    nc.sync.dma_start(out=wv_sb, in_=t["wv"].ap().rearrange("(ci p) d -> p ci d", p=P))
    for kt in range(NKT):
        pa = cx.ps.tile([P, QC], F32, name="mm", tag="mm")
        pb = cx.ps.tile([P, QC], F32, name="mm", tag="mm")
        for ci in range(NCT):
            nc.tensor.matmul(pa, lhsT=imgT[:, ci, kt * P:(kt + 1) * P],
                             rhs=wv_sb[:, ci, :QC], start=(ci == 0), stop=(ci == NCT - 1))
            nc.tensor.matmul(pb[:, :H - QC], lhsT=imgT[:, ci, kt * P:(kt + 1) * P],
                             rhs=wv_sb[:, ci, QC:], start=(ci == 0), stop=(ci == NCT - 1))
        nc.scalar.activation(v[:, kt, 0:8, 0:HD],
                             pa.rearrange("p (h d) -> p h d", d=HD), AF.Copy)
        nc.scalar.activation(v[:, kt, 8:NH, 0:HD],
                             pb[:, :H - QC].rearrange("p (h d) -> p h d", d=HD), AF.Copy)
        nc.vector.tensor_copy(v[:, kt, :, HD], cx.ones_col.to_broadcast((P, NH)))

    # qT[do,q] = sum_c Wq[c,do] * freqT[c,q]   (kT likewise from imgT)
    qT = _rtile(cx, [P, NCT, SQ], F32R, "qT")
    kT = _rtile(cx, [P, NCT, SK], F32R, "kT")
    for nm, src, dst, bb in (("wkr", imgT, kT, _BK), ("wqr", freqT, qT, _BQ)):
        for di in range(NCT):
            slab = cx.wstream.tile([P, NCT, P], F32R, name="wqk", tag="wqk")
            nc.sync.dma_start(out=slab, in_=t[nm].ap()[di])
            for qc in range(NQC):
                pt = cx.ps.tile([P, QC], F32, name="mm", tag="mm")
                for ci in range(NCT):
                    nc.tensor.matmul(
                        pt, lhsT=slab[:, ci, :], rhs=src[:, ci, qc * QC:(qc + 1) * QC],
                        start=(ci == 0), stop=(ci == NCT - 1))
                dst_s = dst[:, di, qc * QC:(qc + 1) * QC]
                if cx.zero.get(bb, False):
                    nc.scalar.activation(dst_s, pt, AF.Copy)
                else:
                    nc.vector.tensor_scalar_add(dst_s, pt, bpc(bb, di))

    # ---- attention -------------------------------------------------------
    ctxT = _rtile(cx, [P, NCT, SQ], F32R, "ctxT")
    est = _rtile(cx, [P, 2, NKT, AC], F32R, "est")   # manual double buffer
    chunk = 0
    for h in range(NH):
        ci, lo = h // 2, (h % 2) * HD
        for ac in range(NAC):
            par = chunk % 2
            chunk += 1
            qs = qT[lo:lo + HD, ci, ac * AC:(ac + 1) * AC]
            for g in range(2):
                sp = cx.score_ps.tile([P, 4, AC], F32, name="sp", tag="sp")
                for j in range(4):
                    kt = 4 * g + j
                    nc.tensor.matmul(sp[:, j, :],
                                     lhsT=kT[lo:lo + HD, ci, kt * P:(kt + 1) * P],
                                     rhs=qs, start=True, stop=True)
                nc.scalar.activation(est[:, par, 4 * g:4 * g + 4, :], sp, AF.Exp)
            cp = cx.ctx_ps.tile([HD + 1, AC], F32, name="cp", tag="cp")
            for kt in range(NKT):
                nc.tensor.matmul(cp, lhsT=v[:, kt, h, :], rhs=est[:, par, kt, :],
                                 start=(kt == 0), stop=(kt == NKT - 1))
            rec = cx.stats.tile([1, AC], F32R, name="rec", tag="stat")
            with nc.allow_low_precision(reason="f32r recip feeds PE broadcast"):
                nc.vector.reciprocal(rec, cp[HD:HD + 1, :])
            rb = cx.ps.tile([HD, AC], F32, name="mm", tag="mm")
            nc.tensor.matmul(rb, lhsT=cx.ones_row[:, 0:HD], rhs=rec,
                             start=True, stop=True)
            cslice = ctxT[lo:lo + HD, ci, ac * AC:(ac + 1) * AC]
            nc.vector.tensor_copy(cslice, cp[0:HD, :])
            nc.vector.tensor_tensor(cslice, cslice, rb, OP.mult)
            if not cx.zero.get(_BV, False):
                nc.vector.tensor_scalar_add(cslice, cslice, bpc(_BV, ci, lo, HD))

    # ---- h = freqT + LN(ctxT) -------------------------------------------
    freqT2 = _rtile(cx, [P, NCT, SQ], F32R, "freqT2")
    nc.sync.dma_start(out=freqT2,
                      in_=t["freqT"].ap().rearrange("(ci p) q -> p ci q", p=P))
    hT = _rtile(cx, [P, NCT, SQ], F32R, "hT")
    for qc in range(NQC):
        _emit_ln_residual(nc, cx, ctxT[:, :, qc * QC:(qc + 1) * QC],
                          freqT2[:, :, qc * QC:(qc + 1) * QC],
                          hT[:, :, qc * QC:(qc + 1) * QC], _G1, _BE1)

    # ---- MLP (bf16 weights/acts, fp32 PSUM) + second LN ------------------
    hB = _rtile(cx, [P, NCT, SQ], BF16, "hB")
    nc.vector.tensor_copy(hB, hT)
    for qc in range(NQC):
        m1 = _rtile(cx, [P, NFT, QC], BF16, "m1")
        m2o = _rtile(cx, [P, 2, NCT, QC], F32R, "m2o")  # [0]=mlp2, [1]=out
        for fi in range(NFT):
            slab = cx.wstream.tile([P, NCT, P], BF16, name="w1", tag="w1")
            nc.sync.dma_start(out=slab, in_=t["w1r"].ap()[fi])
            pt = cx.ps.tile([P, QC], F32, name="mm", tag="mm")
            for ci in range(NCT):
                nc.tensor.matmul(pt, lhsT=slab[:, ci, :],
                                 rhs=hB[:, ci, qc * QC:(qc + 1) * QC],
                                 start=(ci == 0), stop=(ci == NCT - 1))
            # bias + relu on the (otherwise idle) ACT engine
            nc.scalar.activation(m1[:, fi, :], pt, AF.Relu, bias=bpc(_B1, fi))
        for ci in range(NCT):
            slab = cx.wstream.tile([P, NFT, P], BF16, name="w2", tag="w2")
            nc.sync.dma_start(out=slab, in_=t["w2r"].ap()[ci])
            pt = cx.ps.tile([P, QC], F32, name="mm", tag="mm")
            for fi in range(NFT):
                nc.tensor.matmul(pt, lhsT=slab[:, fi, :], rhs=m1[:, fi, :],
                                 start=(fi == 0), stop=(fi == NFT - 1))
            if cx.zero.get(_B2, False):
                nc.scalar.activation(m2o[:, 0, ci, :], pt, AF.Copy)
            else:
                nc.vector.tensor_scalar_add(m2o[:, 0, ci, :], pt, bpc(_B2, ci))
        _emit_ln_residual(nc, cx, m2o[:, 0], hT[:, :, qc * QC:(qc + 1) * QC],
                          m2o[:, 1], _G2, _BE2)
        nc.sync.dma_start(
            out=t["outT"].ap().rearrange("(ci p) q -> p ci q", p=P)[:, :, qc * QC:(qc + 1) * QC],
            in_=m2o[:, 1])


def _emit_ln_residual(nc, cx, xs, rs, os_, gbase, bbase):
    """os_ = rs + layernorm(xs)*g + b over features (partition x ci dim).

    xs/rs/os_: [P, NCT, W] APs (W tokens). Mean/var via ones-matmul partition
    reductions; per-token stats broadcast back with a rank-1 ones matmul.
    """
    W = xs.shape[2]
    bp = cx.bp
    sq = _rtile(cx, [P, NCT, W], F32R, "ln_sq")
    nc.vector.tensor_tensor(sq, xs, xs, OP.mult)
    s_ps = cx.ps.tile([1, W], F32, name="mm", tag="mm")
    for ci in range(NCT):
        nc.tensor.matmul(s_ps, lhsT=cx.ones_col, rhs=xs[:, ci, :],
                         start=(ci == 0), stop=(ci == NCT - 1))
    q_ps = cx.ps.tile([1, W], F32, name="mm", tag="mm")
    for ci in range(NCT):
        nc.tensor.matmul(q_ps, lhsT=cx.ones_col, rhs=sq[:, ci, :],
                         start=(ci == 0), stop=(ci == NCT - 1))
    m = cx.stats.tile([1, W], F32R, name="ln_m", tag="stat")
    nc.vector.tensor_scalar_mul(m, s_ps, 1.0 / H)
    var = cx.stats.tile([1, W], F32, name="ln_var", tag="stat")
    nc.vector.tensor_tensor(var, m, m, OP.mult)
    ex2 = cx.stats.tile([1, W], F32, name="ln_ex2", tag="stat")
    nc.vector.tensor_scalar_mul(ex2, q_ps, 1.0 / H)
    nc.vector.tensor_tensor(var, ex2, var, OP.subtract)
    std = cx.stats.tile([1, W], F32, name="ln_std", tag="stat")
    nc.scalar.activation(std, var, AF.Sqrt, bias=cx.eps)
    inv = cx.stats.tile([1, W], F32R, name="ln_inv", tag="stat")
    with nc.allow_low_precision(reason="f32r recip feeds PE broadcast"):
        nc.vector.reciprocal(inv, std)
    mb = cx.ps.tile([P, W], F32, name="mm", tag="mm")
    nc.tensor.matmul(mb, lhsT=cx.ones_row, rhs=m, start=True, stop=True)
    ib = cx.ps.tile([P, W], F32, name="mm", tag="mm")
    nc.tensor.matmul(ib, lhsT=cx.ones_row, rhs=inv, start=True, stop=True)
    plain = cx.zero.get(gbase, False) and cx.zero.get(bbase, False)
    for ci in range(NCT):
        # GpSimd can't read PSUM, so DVE does the (x-mb)*ib part for all ci;
        # the SBUF-only residual add alternates DVE/GpSimd to split the load.
        tt = cx.scratch.tile([P, W], F32, name="lnt", tag="lnt")
        nc.vector.tensor_tensor(tt, xs[:, ci, :], mb, OP.subtract)
        nc.vector.tensor_tensor(tt, tt, ib, OP.mult)
        if not plain:
            nc.vector.tensor_scalar(out=tt, in0=tt,
                                    scalar1=bp[:, gbase + ci:gbase + ci + 1],
                                    scalar2=bp[:, bbase + ci:bbase + ci + 1],
                                    op0=OP.mult, op1=OP.add)
        eng = nc.gpsimd if ci % 2 else nc.vector
        eng.tensor_tensor(os_[:, ci, :], tt, rs[:, ci, :], OP.add)


def build_nc(reps=1, zero=None):
    nc = bacc.Bacc("TRN2", target_bir_lowering=False, debug=False)
    t = {}
    t["freqT"] = nc.declare_dram_parameter("freqT", [H, SQ], F32R, isOutput=False)
    t["imgT"] = nc.declare_dram_parameter("imgT", [H, SK], F32R, isOutput=False)
    t["wqr"] = nc.declare_dram_parameter("wqr", [NCT, P, NCT, P], F32R, isOutput=False)
    t["wkr"] = nc.declare_dram_parameter("wkr", [NCT, P, NCT, P], F32R, isOutput=False)
    t["wv"] = nc.declare_dram_parameter("wv", [H, H], F32R, isOutput=False)
    t["w1r"] = nc.declare_dram_parameter("w1r", [NFT, P, NCT, P], BF16, isOutput=False)
    t["w2r"] = nc.declare_dram_parameter("w2r", [NCT, P, NFT, P], BF16, isOutput=False)
    t["bias_pack"] = nc.declare_dram_parameter("bias_pack", [P, _BP_COLS], F32, isOutput=False)
    t["ones_col"] = nc.declare_dram_parameter("ones_col", [P, 1], F32R, isOutput=False)
    t["ones_row"] = nc.declare_dram_parameter("ones_row", [1, P], F32R, isOutput=False)
    t["outT"] = nc.declare_dram_parameter("outT", [H, SQ], F32R, isOutput=True)

    with tile.TileContext(nc) as tc:
        _emit(nc, tc, t, reps=reps, zero=zero)
    nc.compile()
    return nc


def _host_prep(inputs):
    f = np.ascontiguousarray(np.asarray(inputs["freq_hidden_states"], np.float32))
    im = np.ascontiguousarray(np.asarray(inputs["image_hidden_states"], np.float32))
    scale = np.float32(1.0 / np.sqrt(np.float32(HD)))
    wq = np.asarray(inputs["Wq"], np.float32) * scale
    bq = np.asarray(inputs["bq"], np.float32) * scale
    wk = np.asarray(inputs["Wk"], np.float32)
    wv = np.ascontiguousarray(np.asarray(inputs["Wv"], np.float32))
    w1 = np.asarray(inputs["W1"], np.float32)
    w2 = np.asarray(inputs["W2"], np.float32)

    def qk_tiles(w):  # [768,768] -> [di, p, ci, 128] holding lhsT tiles w[ci,di]
        return np.ascontiguousarray(w.reshape(NCT, P, NCT, P).transpose(2, 1, 0, 3))

    wqr = qk_tiles(wq)
    wkr = qk_tiles(wk)
    import ml_dtypes
    w1r = np.ascontiguousarray(
        w1.reshape(NCT, P, NFT, P).transpose(2, 1, 0, 3)).astype(ml_dtypes.bfloat16)
    w2r = np.ascontiguousarray(
        w2.reshape(NFT, P, NCT, P).transpose(2, 1, 0, 3)).astype(ml_dtypes.bfloat16)

    bpk = np.zeros((P, _BP_COLS), np.float32)
    for base, vec in ((_BQ, bq), (_BK, np.asarray(inputs["bk"], np.float32)),
                      (_BV, np.asarray(inputs["bv"], np.float32)),
                      (_B2, np.asarray(inputs["b2"], np.float32)),
                      (_G1, np.asarray(inputs["ln_att_g"], np.float32)),
                      (_BE1, np.asarray(inputs["ln_att_b"], np.float32)),
                      (_G2, np.asarray(inputs["ln_mlp_g"], np.float32)),
                      (_BE2, np.asarray(inputs["ln_mlp_b"], np.float32)),
                      (_B1, np.asarray(inputs["b1"], np.float32))):
        n = vec.shape[0] // P
        bpk[:, base:base + n] = vec.reshape(n, P).T
    shared = {
        "wqr": wqr, "wkr": wkr, "wv": wv, "w1r": w1r, "w2r": w2r,
        "bias_pack": bpk,
        "ones_col": np.ones((P, 1), np.float32),
        "ones_row": np.ones((1, P), np.float32),
    }
    in_maps = []
    for i in range(B):
        m = dict(shared)
        m["freqT"] = np.ascontiguousarray(f[i].T)
        m["imgT"] = np.ascontiguousarray(im[i].T)
        in_maps.append(m)
    return in_maps


_NC_CACHE = {}


def _zero_flags(inputs):
    def z(v):
        return bool(np.all(np.asarray(v) == 0))

    def one(v):
        return bool(np.all(np.asarray(v) == 1))

    return {
        _BQ: z(inputs["bq"]), _BK: z(inputs["bk"]), _BV: z(inputs["bv"]),
        _B2: z(inputs["b2"]),
        _G1: one(inputs["ln_att_g"]), _BE1: z(inputs["ln_att_b"]),
        _G2: one(inputs["ln_mlp_g"]), _BE2: z(inputs["ln_mlp_b"]),
    }


def kernel(**inputs) -> np.ndarray:
    in_maps = _host_prep(inputs)
    key = tuple(sorted(_zero_flags(inputs).items()))
    if _NC_CACHE.get("key") != key:
        _NC_CACHE["nc"] = build_nc(zero=_zero_flags(inputs))
        _NC_CACHE["key"] = key
    nc = _NC_CACHE["nc"]
    res = run_bass_kernel_spmd(nc, in_maps, core_ids=list(range(B)))
    out = np.stack([np.ascontiguousarray(r["outT"].T) for r in res.results])
    return out.astype(np.float32)
